# revision 28
# baseline (speedup 1.0000x reference)
"""Trainium2 Bass kernel for nn_Model_51144470560940 (moe_routing).

Sharding: batch 32 -> 8 cores x 4. Per core tokens = 4*321 = 1284 (c-major:
token t = c*4 + b). Activations feature-major [features->partitions,
tokens->free], 3 token chunks of 428.

v2: interleaved schedule. Per-chunk DDI scans overlap the next chunk's MDM
and the expert matmuls of earlier chunks; gating is batched across all 12
token tiles (2 act-table loads instead of 36); expert path runs in bf16
with all 4 experts' weights resident in SBUF; expert combine uses a
PE row-broadcast of the gates.
"""
import numpy as np
from contextlib import ExitStack

import concourse.bass as bass
import concourse.tile as tile
from concourse import bacc, mybir
from concourse.bass_utils import run_bass_kernel_spmd
from concourse.masks import make_identity

F32 = mybir.dt.float32
F32R = mybir.dt.float32r
BF16 = mybir.dt.bfloat16
AF = mybir.ActivationFunctionType
ALU = mybir.AluOpType
AX = mybir.AxisListType

B, L, C, P_OUT = 32, 336, 321, 96
E, H = 4, 2048
NCORE = 8
BPC = B // NCORE            # 4 batches per core
T = C * BPC                 # 1284 tokens per core
NCH = 3
TCH = T // NCH              # 428 tokens per chunk
CCH = C // NCH              # 107 c's per chunk
PATCH, NPAT = 12, 28
FOLD, FW = 10, 44           # fold groups x width per chunk
PF = FOLD * PATCH           # 120
INV = float(1.0 / np.sqrt(1.0 + 1e-5))
ALPHA = 10.0
D0, D1, D2 = 42, 84, 168    # MDM scale dims (L//8, L//4, L//2)
NTT = NCH * 4               # 12 token tiles of <=128 for gating

_CACHE = {}


def _pad_k(w):
    """[k, m] -> [ceil(k/128), 128, m] zero-padded along k."""
    k, m = w.shape
    kt = -(-k // 128)
    out = np.zeros((kt, 128, m), np.float32)
    out.reshape(kt * 128, m)[:k] = w
    return out


def _bias_cols(b):
    """[m] -> [128, ceil(m/128)] (column mt = partitions of m-tile mt)."""
    m = b.shape[0]
    mt = -(-m // 128)
    out = np.zeros((mt * 128,), np.float32)
    out[:m] = b
    return np.ascontiguousarray(out.reshape(mt, 128).T)


def build_host_weights(inp):
    """Preprocess weights into device layouts (shared across cores)."""
    import ml_dtypes
    w = {}
    for name, src_w, src_b in (
        ("mdm", inp["mdm_bn_w"], inp["mdm_bn_b"]),
        ("ddi", inp["ddi_bn_w"], inp["ddi_bn_b"]),
    ):
        s = (np.asarray(src_w, np.float64) * INV).astype(np.float32).reshape(C, L).T
        t = np.asarray(src_b, np.float32).reshape(C, L).T
        w[f"{name}_s"] = _pad_k(s)
        w[f"{name}_t"] = _pad_k(t)
    pm = np.zeros((L, D0 + D1 + D2), np.float32)
    for i in range(D0):
        pm[8 * i : 8 * i + 8, i] = 0.125
    for i in range(D1):
        pm[4 * i : 4 * i + 4, D0 + i] = 0.25
    for i in range(D2):
        pm[2 * i : 2 * i + 2, D0 + D1 + i] = 0.5
    w["poolmat"] = _pad_k(pm)
    for i in range(3):
        w[f"mw1_{i}"] = _pad_k(np.asarray(inp[f"m{i}w1"], np.float32).T)
        w[f"mw2_{i}"] = _pad_k(np.asarray(inp[f"m{i}w2"], np.float32).T)
        w[f"mb1_{i}"] = _bias_cols(np.asarray(inp[f"m{i}b1"], np.float32))
        w[f"mb2_{i}"] = _bias_cols(np.asarray(inp[f"m{i}b2"], np.float32))
    w["gatew"] = _pad_k(np.asarray(inp["gate_w"], np.float32).T)     # [3,128,4]
    w["gateb"] = np.asarray(inp["gate_b"], np.float32).reshape(1, 4)
    bd = np.kron(np.eye(FOLD, dtype=np.float32),
                 np.asarray(inp["agg_w"], np.float32).T)
    w["bd"] = _pad_k(bd)                                             # [1,128,120]
    w["aggb"] = _bias_cols(np.tile(np.asarray(inp["agg_b"], np.float32), FOLD))
    # n1 scale/shift in folded layout: [128, NCH*FW], row g*12+j, col ch*FW+c
    n1s = (np.asarray(inp["ddi_n1_w"], np.float64) * INV).astype(np.float32).reshape(C, PATCH)
    n1t = np.asarray(inp["ddi_n1_b"], np.float32).reshape(C, PATCH)
    fs = np.zeros((128, NCH * FW), np.float32)
    ft = np.zeros((128, NCH * FW), np.float32)
    for ch in range(NCH):
        for g in range(FOLD):
            for c in range(FW):
                tl = g * FW + c
                if tl >= TCH:
                    continue
                cg = (ch * TCH + tl) // BPC
                fs[g * PATCH : (g + 1) * PATCH, ch * FW + c] = n1s[cg]
                ft[g * PATCH : (g + 1) * PATCH, ch * FW + c] = n1t[cg]
    w["n1sf"], w["n1tf"] = fs, ft
    ew1 = np.asarray(inp["ew1"], np.float32)                         # [E,H,L]
    w["ew1"] = np.stack([_pad_k(ew1[e].T) for e in range(E)]).astype(
        ml_dtypes.bfloat16)                                          # [E,3,128,H]
    w["eb1"] = np.stack([_bias_cols(np.asarray(inp["eb1"], np.float32)[e])
                         for e in range(E)])                         # [E,128,16]
    ew2 = np.asarray(inp["ew2"], np.float32)                         # [E,P,H]
    w["ew2"] = np.stack([np.ascontiguousarray(ew2[e].T.reshape(16, 128, P_OUT))
                         for e in range(E)]).astype(ml_dtypes.bfloat16)
    w["eb2"] = np.stack([_bias_cols(np.asarray(inp["eb2"], np.float32)[e])
                         for e in range(E)])                         # [E,128,1]
    return w


def build_nc(debug=False):
    nc = bacc.Bacc("TRN2", target_bir_lowering=False, debug=False)
    d = {}

    def din(name, shape, dt=F32):
        d[name] = nc.dram_tensor(name, list(shape), dt, kind="ExternalInput")

    din("xT", (3, 128, T))
    for n in ("mdm_s", "mdm_t", "ddi_s", "ddi_t"):
        din(n, (3, 128, C))
    din("poolmat", (3, 128, D0 + D1 + D2))
    din("mw1_0", (1, 128, D0)); din("mw2_0", (1, 128, D1))
    din("mw1_1", (1, 128, D1)); din("mw2_1", (1, 128, D2))
    din("mw1_2", (2, 128, D2)); din("mw2_2", (2, 128, L))
    din("mb1_0", (128, 1)); din("mb2_0", (128, 1))
    din("mb1_1", (128, 1)); din("mb2_1", (128, 2))
    din("mb1_2", (128, 2)); din("mb2_2", (128, 3))
    din("gatew", (3, 128, 4))
    din("gateb", (1, 4))
    din("bd", (1, 128, PF), F32R)
    din("aggb", (128, 1))
    din("n1sf", (128, NCH * FW)); din("n1tf", (128, NCH * FW))
    din("ew1", (E, 3, 128, H), BF16)
    din("eb1", (E, 128, 16))
    din("ew2", (E, 16, 128, P_OUT), BF16)
    din("eb2", (E, 128, 1))
    out_d = nc.dram_tensor("outT", [P_OUT, T], F32, kind="ExternalOutput")
    dbg = {}
    if debug:
        for n, shape, dt_ in (("dbg_te", (3, 128, T), F32),
                              ("dbg_ddi", (3, 128, T), BF16),
                              ("dbg_gates", (4, T), F32)):
            dbg[n] = nc.dram_tensor(n, list(shape), dt_, kind="ExternalOutput")

    with tile.TileContext(nc) as tc, ExitStack() as ctx:
        const = ctx.enter_context(tc.tile_pool(name="const", bufs=1))
        io_p = ctx.enter_context(tc.tile_pool(name="io", bufs=1))
        fold_p = ctx.enter_context(tc.tile_pool(name="fold", bufs=2))
        wk = ctx.enter_context(tc.tile_pool(name="wk", bufs=1))
        sc = ctx.enter_context(tc.tile_pool(name="sc", bufs=2))
        hid_p = ctx.enter_context(tc.tile_pool(name="hid", bufs=1))
        ps_mdm = ctx.enter_context(tc.tile_pool(name="psmdm", bufs=2, space="PSUM"))
        ps_sm = ctx.enter_context(tc.tile_pool(name="pssm", bufs=2, space="PSUM"))
        ps_mm1 = ctx.enter_context(tc.tile_pool(name="psmm1", bufs=2, space="PSUM"))
        ps_mm2 = ctx.enter_context(tc.tile_pool(name="psmm2", bufs=2, space="PSUM"))
        dram = ctx.enter_context(tc.tile_pool(name="dram", bufs=1, space="DRAM"))

        # ---- input on the scalar queue (idle until first gelu), consts on
        # sync, ddi-side consts on gpsimd ----
        xfull = io_p.tile([128, 3, T], F32, name="xfull")
        nc.scalar.dma_start(xfull[:], d["xT"].ap()[:].rearrange("k p t -> p k t"))
        cw = {}
        for n in ("mdm_s", "mdm_t"):
            cw[n] = const.tile([128, 3, C], F32, name=n)
            nc.sync.dma_start(cw[n][:], d[n].ap()[:].rearrange("k p c -> p k c"))
        pool_w = const.tile([128, 3, D0 + D1 + D2], F32, name="poolmat")
        nc.sync.dma_start(pool_w[:], d["poolmat"].ap()[:].rearrange("k p m -> p k m"))
        for n in ("mw1_0", "mw2_0", "mw1_1", "mw2_1", "mw1_2", "mw2_2"):
            kt = d[n].shape[0]
            cw[n] = const.tile([128, kt, d[n].shape[2]], F32, name=n)
            nc.sync.dma_start(cw[n][:], d[n].ap()[:].rearrange("k p m -> p k m"))
        for n in ("mb1_0", "mb2_0", "mb1_1", "mb2_1", "mb1_2", "mb2_2"):
            cw[n] = const.tile(list(d[n].shape), F32, name=n)
            nc.sync.dma_start(cw[n][:], d[n].ap()[:])
        gate_w = const.tile([128, 3, 4], F32, name="gatew")
        nc.sync.dma_start(gate_w[:], d["gatew"].ap()[:].rearrange("k p m -> p k m"))
        gateb_row = const.tile([1, 4], F32, name="gateb_row")
        nc.sync.dma_start(gateb_row[:], d["gateb"].ap()[:])
        for n in ("ddi_s", "ddi_t"):
            cw[n] = const.tile([128, 3, C], F32, name=n)
            nc.gpsimd.dma_start(cw[n][:], d[n].ap()[:].rearrange("k p c -> p k c"))
        cw["bd"] = const.tile([128, 1, PF], F32R, name="bd")
        nc.gpsimd.dma_start(cw["bd"][:], d["bd"].ap()[:].rearrange("k p m -> p k m"))
        for n in ("aggb", "n1sf", "n1tf"):
            cw[n] = const.tile(list(d[n].shape), F32, name=n)
            nc.gpsimd.dma_start(cw[n][:], d[n].ap()[:])
        gateb_bc = const.tile([128, 4], F32, name="gateb_bc")
        nc.gpsimd.partition_broadcast(gateb_bc[:], gateb_row[:])
        negalpha = const.tile([128, 1], F32, name="negalpha")
        nc.vector.memset(negalpha[:], -ALPHA)
        ident = const.tile([128, 128], F32, name="ident")
        make_identity(nc, ident[:])
        ones96 = const.tile([128, P_OUT], F32, name="ones96")
        nc.vector.memset(ones96[:], 1.0)
        # expert weights: all resident, loaded via sync DMA queue
        ew1_t, eb1_t, ew2_t, eb2_t = [], [], [], []
        for e in range(E):
            w1 = const.tile([128, 3, H], BF16, name=f"ew1_{e}")
            nc.sync.dma_start(w1[:], d["ew1"].ap()[e].rearrange("k p m -> p k m"))
            b1 = const.tile([128, 16], F32, name=f"eb1_{e}")
            nc.sync.dma_start(b1[:], d["eb1"].ap()[e])
            w2 = const.tile([128, 16, P_OUT], BF16, name=f"ew2_{e}")
            nc.sync.dma_start(w2[:], d["ew2"].ap()[e].rearrange("k p m -> p k m"))
            b2 = const.tile([128, 1], F32, name=f"eb2_{e}")
            nc.sync.dma_start(b2[:], d["eb2"].ap()[e])
            ew1_t.append(w1); eb1_t.append(b1); ew2_t.append(w2); eb2_t.append(b2)

        # ---- persistent tiles ----
        ddi_out = io_p.tile([128, 3, T], BF16, name="ddi_out")
        nc.vector.memset(ddi_out[:, 2, :].bitcast(F32), 0.0)
        gates_fm4 = io_p.tile([4, T], F32, name="gates_fm4")
        gates_fm = io_p.tile([128, T], F32, name="gates_fm")
        z_all = io_p.tile([128, NTT, 4], F32, name="z_all")
        nc.vector.memset(z_all[:], 0.0)
        acc = [io_p.tile([P_OUT, TCH], F32, name=f"acc{ch}") for ch in range(NCH)]
        dscr = dram.tile([L, T], F32R, name="dscr")
        oscr = [dram.tile([L, TCH], BF16, name=f"oscr{ch}") for ch in range(NCH)]

        # MDM persistent-by-name work tiles (memset tails once; reused per chunk)
        s0 = wk.tile([128, TCH], F32, name="s0")
        s1 = wk.tile([128, TCH], F32, name="s1")
        s2 = wk.tile([128, 2, TCH], F32, name="s2")
        a0 = wk.tile([128, TCH], F32, name="a0")
        cur1 = wk.tile([128, TCH], F32, name="cur1")
        cur2 = wk.tile([128, 2, TCH], F32, name="cur2")
        a2 = wk.tile([128, 2, TCH], F32, name="a2")
        te = wk.tile([128, 3, TCH], F32, name="te")
        nc.vector.memset(s0[:], 0.0)
        nc.vector.memset(s1[:], 0.0)
        nc.vector.memset(s2[:, 1, :], 0.0)
        nc.vector.memset(a0[:], 0.0)
        nc.vector.memset(cur1[:], 0.0)
        nc.vector.memset(cur2[:, 1, :], 0.0)
        nc.vector.memset(a2[:, 1, :], 0.0)
        nc.vector.memset(te[:, 2, :], 0.0)

        # =========== stage builders ===========
        def mdm_stages(ch):
            tsl = slice(ch * TCH, (ch + 1) * TCH)
            csl = slice(ch * CCH, (ch + 1) * CCH)
            st = []

            def bn_into(dst, src, sname, tname):
                # Pool engine: keeps the DVE free for the scan chains
                for k in range(3):
                    sB = cw[sname][:, k, csl].unsqueeze(2).broadcast_to([128, CCH, BPC])
                    tB = cw[tname][:, k, csl].unsqueeze(2).broadcast_to([128, CCH, BPC])
                    xv = src[:, k, :].rearrange("p (c b) -> p c b", b=BPC)
                    hv = dst[:, k, :].rearrange("p (c b) -> p c b", b=BPC)
                    nc.gpsimd.tensor_tensor(hv, xv, sB, op=ALU.mult)
                    nc.gpsimd.tensor_tensor(hv, hv, tB, op=ALU.add)

            h = wk.tile([128, 3, TCH], F32, name="h")

            def st_bn():
                bn_into(h, xfull[:, :, tsl], "mdm_s", "mdm_t")
            st.append(st_bn)

            for m0, m1 in ((0, D0), (D0, D0 + D1), (D0 + D1, D0 + D1 + 128),
                           (D0 + D1 + 128, D0 + D1 + D2)):
                def st_pool(m0=m0, m1=m1):
                    if m0 == 0:
                        dst = s0[:D0, :]
                    elif m0 == D0:
                        dst = s1[:D1, :]
                    elif m0 == D0 + D1:
                        dst = s2[:128, 0, :]
                    else:
                        dst = s2[: D2 - 128, 1, :]
                    pp = ps_mdm.tile([128, TCH], F32, name="pmdm")
                    for k in range(3):
                        nc.tensor.matmul(pp[: m1 - m0, :], pool_w[:, k, m0:m1],
                                         h[:, k, :], start=(k == 0), stop=(k == 2))
                    nc.vector.tensor_copy(dst, pp[: m1 - m0, :])
                st.append(st_pool)

            def st_mlp0():
                pa = ps_mdm.tile([128, TCH], F32, name="pmdm")
                nc.tensor.matmul(pa[:D0, :], cw["mw1_0"][:, 0, :], s0[:], start=True, stop=True)
                nc.scalar.activation(a0[:D0, :], pa[:D0, :], AF.Gelu, bias=cw["mb1_0"][:D0, 0:1])
                pt = ps_mdm.tile([128, TCH], F32, name="pmdm")
                nc.tensor.matmul(pt[:D1, :], cw["mw2_0"][:, 0, :], a0[:], start=True, stop=True)
                nc.vector.scalar_tensor_tensor(cur1[:D1, :], pt[:D1, :], cw["mb2_0"][:D1, 0:1],
                                               s1[:D1, :], op0=ALU.add, op1=ALU.add)
            st.append(st_mlp0)

            def st_mlp1():
                pa = ps_mdm.tile([128, TCH], F32, name="pmdm")
                nc.tensor.matmul(pa[:D1, :], cw["mw1_1"][:, 0, :], cur1[:], start=True, stop=True)
                a1 = a0
                nc.scalar.activation(a1[:D1, :], pa[:D1, :], AF.Gelu, bias=cw["mb1_1"][:D1, 0:1])
                for mt, (p0, pk) in enumerate(((0, 128), (128, D2 - 128))):
                    pt = ps_mdm.tile([128, TCH], F32, name="pmdm")
                    nc.tensor.matmul(pt[:pk, :], cw["mw2_1"][:, 0, p0 : p0 + pk], a1[:],
                                     start=True, stop=True)
                    nc.vector.scalar_tensor_tensor(cur2[:pk, mt, :], pt[:pk, :],
                                                   cw["mb2_1"][:pk, mt : mt + 1],
                                                   s2[:pk, mt, :], op0=ALU.add, op1=ALU.add)
            st.append(st_mlp1)

            def st_mlp2a():
                for mt, (p0, pk) in enumerate(((0, 128), (128, D2 - 128))):
                    pa = ps_mdm.tile([128, TCH], F32, name="pmdm")
                    for k in range(2):
                        nc.tensor.matmul(pa[:pk, :], cw["mw1_2"][:, k, p0 : p0 + pk],
                                         cur2[:, k, :], start=(k == 0), stop=(k == 1))
                    nc.scalar.activation(a2[:pk, mt, :], pa[:pk, :], AF.Gelu,
                                         bias=cw["mb1_2"][:pk, mt : mt + 1])
            st.append(st_mlp2a)

            def st_mlp2b():
                for mt in range(3):
                    pk = 128 if mt < 2 else L - 256
                    pt = ps_mdm.tile([128, TCH], F32, name="pmdm")
                    for k in range(2):
                        nc.tensor.matmul(pt[:pk, :], cw["mw2_2"][:, k, 128 * mt : 128 * mt + pk],
                                         a2[:, k, :], start=(k == 0), stop=(k == 1))
                    nc.vector.scalar_tensor_tensor(te[:pk, mt, :], pt[:pk, :],
                                                   cw["mb2_2"][:pk, mt : mt + 1],
                                                   h[:pk, mt, :], op0=ALU.add, op1=ALU.add)
                if debug:
                    for k in range(3):
                        nc.sync.dma_start(dbg["dbg_te"].ap()[k, :, tsl], te[:, k, :])
            st.append(st_mlp2b)

            def st_logits():
                for mt in range(4):
                    t0_ = mt * 128
                    tw = min(128, TCH - t0_)
                    pz = ps_sm.tile([128, 128], F32, name="pssm")
                    for k in range(3):
                        nc.tensor.matmul(pz[:tw, :4], te[:, k, t0_ : t0_ + tw],
                                         gate_w[:, k, :], start=(k == 0), stop=(k == 2))
                    nc.vector.tensor_tensor(z_all[:tw, ch * 4 + mt, :], pz[:tw, :4],
                                            gateb_bc[:tw, :], op=ALU.add)
            st.append(st_logits)

            def st_ddibn():
                dt_ = h  # reuse h buffer: h is dead after st_mlp2b consumed it
                bn_into(dt_, te, "ddi_s", "ddi_t")
                for k, pk in ((0, 128), (1, 128), (2, L - 256)):
                    nc.sync.dma_start(dscr[128 * k : 128 * k + pk, tsl],
                                      dt_[:pk, k, :].bitcast(F32R))
            st.append(st_ddibn)
            return st

        def scan_stages(ch):
            """DDI fold + sequential scan + unfold for one chunk."""
            tsl = slice(ch * TCH, (ch + 1) * TCH)
            fsl = slice(ch * FW, (ch + 1) * FW)
            dF = fold_p.tile([128, NPAT, FW], F32R, name="dF")
            xpn = fold_p.tile([128, NPAT, FW], F32R, name="xpn")
            outB = fold_p.tile([128, NPAT, FW], BF16, name="outB")
            # per-chunk scratch: chunks' scans run concurrently
            t1 = wk.tile([128, FW], F32R, name=f"t1_{ch}")
            t2g = wk.tile([128, FW], F32R, name=f"t2g_{ch}")
            st = []

            def st_fold():
                # zero tails that matmuls/vector ops read but gathers don't
                # write (partition bases must be 32-aligned -> full memsets)
                nc.vector.memset(dF[:].bitcast(F32), 0.0)
                nc.vector.memset(t1[:].bitcast(F32), 0.0)
                nc.vector.memset(xpn[:].bitcast(F32), 0.0)
                for g in range(FOLD):
                    cw_ = min(FW, TCH - g * FW)
                    src = dscr[:, ch * TCH + g * FW : ch * TCH + g * FW + cw_].rearrange(
                        "(s j) c -> j s c", j=PATCH)
                    nc.sync.dma_start(dF[g * PATCH : (g + 1) * PATCH, :, :cw_], src)
            st.append(st_fold)

            def st_xpn():
                n1sB = cw["n1sf"][:PF, fsl].unsqueeze(1).broadcast_to([PF, NPAT, FW])
                n1tB = cw["n1tf"][:PF, fsl].unsqueeze(1).broadcast_to([PF, NPAT, FW])
                nc.gpsimd.tensor_tensor(xpn[:PF, :, :], dF[:PF, :, :], n1sB, op=ALU.mult)
                nc.gpsimd.tensor_tensor(xpn[:PF, :, :], xpn[:PF, :, :], n1tB, op=ALU.add)
                nc.gpsimd.tensor_copy(outB[:PF, 0, :], dF[:PF, 0, :])
            st.append(st_xpn)

            for s in range(1, NPAT):
                def st_step(s=s):
                    if s == 1:
                        rhs = xpn[:, 0, :]
                    else:
                        nc.vector.tensor_tensor(t1[:PF, :], t2g[:PF, :],
                                                cw["n1sf"][:PF, fsl], op=ALU.mult)
                        nc.vector.tensor_tensor(t1[:PF, :], t1[:PF, :],
                                                xpn[:PF, s - 1, :], op=ALU.add)
                        rhs = t1[:, :]
                    pd = ps_sm.tile([128, 128], F32, name="pssm")
                    nc.tensor.matmul(pd[:PF, :FW], cw["bd"][:, 0, :], rhs,
                                     start=True, stop=True)
                    nc.scalar.activation(t2g[:PF, :], pd[:PF, :FW], AF.Gelu,
                                         bias=cw["aggb"][:PF, 0:1])
                    nc.vector.tensor_tensor(outB[:PF, s, :], t2g[:PF, :],
                                            dF[:PF, s, :], op=ALU.add)
                st.append(st_step)

            def st_unfold():
                for g in range(FOLD):
                    cw_ = min(FW, TCH - g * FW)
                    dst = oscr[ch][:, g * FW : g * FW + cw_].rearrange(
                        "(s j) c -> j s c", j=PATCH)
                    nc.gpsimd.dma_start(dst, outB[g * PATCH : (g + 1) * PATCH, :, :cw_])
                for k, pk in ((0, 128), (1, 128), (2, L - 256)):
                    nc.gpsimd.dma_start(ddi_out[:pk, k, tsl],
                                        oscr[ch][128 * k : 128 * k + pk, :])
                if debug:
                    for k in range(3):
                        nc.sync.dma_start(dbg["dbg_ddi"].ap()[k, :, tsl],
                                          ddi_out[:, k, tsl])
            st.append(st_unfold)
            return st

        def gating_stages():
            """Batched noisy-top-k gating over all 12 token tiles."""
            st = []

            def sc3(name):
                return sc.tile([128, NTT, 4], F32, name=name)

            def sc1(name):
                return sc.tile([128, NTT], F32, name=name)

            def bcast(t):
                return t[:].unsqueeze(2).broadcast_to([128, NTT, 4])

            gtm = io_p.tile([128, NTT, 4], F32, name="gtm")

            def st_vec1():
                z3 = z_all[:, :, :]
                m1 = sc1("m1")
                nc.vector.reduce_max(m1[:], z3, axis=AX.X)
                ismax = sc3("ismax")
                nc.vector.tensor_tensor(ismax[:], z3, bcast(m1), op=ALU.is_ge)
                cnt = sc1("cnt")
                nc.vector.reduce_sum(cnt[:], ismax[:], axis=AX.X)
                masked = sc3("masked")
                nc.vector.scalar_tensor_tensor(masked[:], ismax[:], -1e30, z3,
                                               op0=ALU.mult, op1=ALU.add)
                m2 = sc1("m2")
                nc.vector.reduce_max(m2[:], masked[:], axis=AX.X)
                ge2 = sc1("ge2")
                nc.vector.tensor_scalar(ge2[:], cnt[:], 2.0, None, op0=ALU.is_ge)
                d12 = sc1("d12")
                nc.vector.tensor_tensor(d12[:], m1[:], m2[:], op=ALU.subtract)
                kth = sc1("kth")
                nc.vector.tensor_tensor(kth[:], d12[:], ge2[:], op=ALU.mult)
                nc.vector.tensor_tensor(kth[:], kth[:], m2[:], op=ALU.add)
                zs = sc3("zs")
                nc.vector.tensor_tensor(zs[:], z3, bcast(m1), op=ALU.subtract)
                ez = sc3("ez")
                nc.scalar.activation(ez[:], zs[:], AF.Exp)
                se = sc1("se")
                nc.vector.reduce_sum(se[:], ez[:], axis=AX.X)
                rse = sc1("rse")
                nc.vector.reciprocal(rse[:], se[:])
                s_ = sc3("s_")
                nc.vector.tensor_tensor(s_[:], ez[:], bcast(rse), op=ALU.mult)
                # softmax(dec) computed without ln (exp-only, one act table):
                # masked experts: exp(A*ln(1+s)) = (1+s)^10 as a polynomial;
                # top-2 experts:  exp(A*(e^s - 1)) = exp(10*e^s - 10).
                # Values stay in [1, e^17.2] -> no max-subtraction needed.
                p1t = sc3("p1t")
                nc.vector.tensor_scalar(p1t[:], s_[:], 1.0, None, op0=ALU.add)
                p2t = sc3("p2t")
                nc.vector.tensor_tensor(p2t[:], p1t[:], p1t[:], op=ALU.mult)
                p4t = sc3("p4t")
                nc.vector.tensor_tensor(p4t[:], p2t[:], p2t[:], op=ALU.mult)
                p8t = sc3("p8t")
                nc.vector.tensor_tensor(p8t[:], p4t[:], p4t[:], op=ALU.mult)
                p10 = sc3("p10")
                nc.vector.tensor_tensor(p10[:], p8t[:], p2t[:], op=ALU.mult)
                u_ = sc3("u_")
                nc.scalar.activation(u_[:], s_[:], AF.Exp)
                w_ = sc3("w_")
                nc.scalar.activation(w_[:], u_[:], AF.Exp, bias=negalpha[:, 0:1],
                                     scale=ALPHA)
                mask = sc3("mask")
                nc.vector.tensor_tensor(mask[:], z3, bcast(kth), op=ALU.is_lt)
                fd = sc3("fd")
                nc.vector.tensor_tensor(fd[:], p10[:], w_[:], op=ALU.subtract)
                f_ = sc3("f_")
                nc.vector.tensor_tensor(f_[:], mask[:], fd[:], op=ALU.mult)
                nc.vector.tensor_tensor(f_[:], f_[:], w_[:], op=ALU.add)
                sx = sc1("sx")
                nc.vector.reduce_sum(sx[:], f_[:], axis=AX.X)
                rsx = sc1("rsx")
                nc.vector.reciprocal(rsx[:], sx[:])
                nc.vector.tensor_tensor(gtm[:], f_[:], bcast(rsx), op=ALU.mult)
            st.append(st_vec1)

            def st_tr():
                for ti in range(NTT):
                    ch, mt = divmod(ti, 4)
                    t0_ = ch * TCH + mt * 128
                    tw = min(128, TCH - mt * 128)
                    ptr = ps_sm.tile([128, 128], F32, name="pssm")
                    nc.tensor.transpose(ptr[:4, :tw], gtm[:tw, ti, :], ident[:tw, :tw])
                    nc.vector.tensor_copy(gates_fm4[0:4, t0_ : t0_ + tw], ptr[:4, :tw])
                nc.sync.dma_start(gates_fm[0:128:32, :], gates_fm4[0:4, :])
                if debug:
                    nc.sync.dma_start(dbg["dbg_gates"].ap()[:], gates_fm[0:128:32, :])
            st.append(st_tr)
            return st

        def expert_stages(ch):
            tsl = slice(ch * TCH, (ch + 1) * TCH)
            st = []
            for e in range(E):
                hid = hid_p.tile([128, 16, TCH], BF16, name="hid")
                for mtg in range(8):
                    def st_mm1(e=e, hid=hid, mtg=mtg):
                        for mt in range(2 * mtg, 2 * mtg + 2):
                            p1 = ps_mm1.tile([128, TCH], F32, name="pmm1")
                            for k in range(3):
                                nc.tensor.matmul(p1[:], ew1_t[e][:, k, 128 * mt : 128 * (mt + 1)],
                                                 ddi_out[:, k, tsl], start=(k == 0), stop=(k == 2))
                            nc.scalar.activation(hid[:, mt, :], p1[:], AF.Gelu,
                                                 bias=eb1_t[e][:, mt : mt + 1])
                    st.append(st_mm1)

                def st_mm2(e=e, hid=hid):
                    p2 = ps_mm2.tile([P_OUT, TCH], F32, name="pmm2")
                    for kt in range(16):
                        nc.tensor.matmul(p2[:], ew2_t[e][:, kt, :], hid[:, kt, :],
                                         start=(kt == 0), stop=(kt == 15))
                    gps = ps_mdm.tile([128, TCH], F32, name="pmdm")
                    nc.tensor.matmul(gps[:P_OUT, :], ones96[32 * e : 32 * e + 1, :],
                                     gates_fm[32 * e : 32 * e + 1, tsl],
                                     start=True, stop=True, tile_position=(32 * e, 0))
                    gbc = sc.tile([P_OUT, TCH], F32, name="gbc")
                    nc.vector.tensor_copy(gbc[:], gps[:P_OUT, :])
                    if e == 0:
                        nc.vector.scalar_tensor_tensor(acc[ch][:], p2[:], eb2_t[e][:P_OUT, 0:1],
                                                       gbc[:], op0=ALU.add, op1=ALU.mult)
                    else:
                        tmp = sc.tile([P_OUT, TCH], F32, name="etmp")
                        nc.vector.scalar_tensor_tensor(tmp[:], p2[:], eb2_t[e][:P_OUT, 0:1],
                                                       gbc[:], op0=ALU.add, op1=ALU.mult)
                        nc.gpsimd.tensor_tensor(acc[ch][:], acc[ch][:], tmp[:], op=ALU.add)
                    if e == E - 1:
                        nc.sync.dma_start(out_d.ap()[:, tsl], acc[ch][:])
                st.append(st_mm2)
            return st

        def weave(a, b):
            """Interleave two stage lists proportionally (a paces, b spreads)."""
            out, i, j = [], 0, 0
            while i < len(a) or j < len(b):
                fa = i / len(a) if a else 1.0
                fb = j / len(b) if b else 1.0
                if i < len(a) and (j >= len(b) or fa <= fb):
                    out.append(a[i]); i += 1
                else:
                    out.append(b[j]); j += 1
            return out

        def run_st(stages):
            for s in stages:
                s()

        def interleave2(a, b):
            """Alternate elements of two lists (chains progress concurrently)."""
            out = []
            for i in range(max(len(a), len(b))):
                if i < len(a):
                    out.append(a[i])
                if i < len(b):
                    out.append(b[i])
            return out

        # =========== schedule ===========
        # Scans are staggered two-deep: scan(ch) overlaps MDM(ch+1) and the
        # tail of scan(ch-1); scan(2) overlaps the chunk-0 expert matmuls.
        s0_ = scan_stages(0)
        s1_ = scan_stages(1)
        s2_ = scan_stages(2)
        run_st(mdm_stages(0))
        run_st(weave(mdm_stages(1), s0_[:16]))
        run_st(weave(mdm_stages(2) + gating_stages(),
                     interleave2(s0_[16:], s1_[:16])))
        # scan(1) must be fully emitted before expert chunk 1 reads ddi_out
        # (tile deps are program-order based), so split the weave.
        run_st(weave(expert_stages(0), weave(s1_[16:], s2_[:22])))
        run_st(weave(expert_stages(1), s2_[22:]))
        run_st(expert_stages(2))

    nc.compile()
    return nc


def _get_nc(debug=False):
    key = "dbg" if debug else "std"
    if key not in _CACHE:
        _CACHE[key] = build_nc(debug)
    return _CACHE[key]


def make_in_maps(inputs):
    w = build_host_weights(inputs)
    x = np.asarray(inputs["x"], np.float32)
    in_maps = []
    for m in range(NCORE):
        xm = x[m * BPC : (m + 1) * BPC]                   # [4, L, C]
        xT = np.zeros((3, 128, T), np.float32)
        xT.reshape(384, T)[:L] = xm.transpose(1, 2, 0).reshape(L, T)
        im = {"xT": xT}
        im.update(w)
        in_maps.append(im)
    return in_maps


def _gather_out(results):
    out = np.empty((B, P_OUT, C), np.float32)
    for m in range(NCORE):
        o = results[m]["outT"]                            # [96, 1284]
        out[m * BPC : (m + 1) * BPC] = o.reshape(P_OUT, C, BPC).transpose(2, 0, 1)
    return out


def run(inputs, debug=False, **kw):
    nc = _get_nc(debug=debug)
    in_maps = make_in_maps(inputs)
    res = run_bass_kernel_spmd(nc, in_maps, core_ids=list(range(NCORE)), **kw)
    return res


def kernel(**inputs) -> np.ndarray:
    res = run(inputs, debug=False)
    return _gather_out(res.results)


# revision 31
# speedup vs baseline: 1.0439x; 1.0439x over previous
"""Trainium2 Bass kernel for nn_Model_51144470560940 (moe_routing).

Sharding: batch 32 -> 8 cores x 4. Per core tokens = 4*321 = 1284 (c-major:
token t = c*4 + b). Activations feature-major [features->partitions,
tokens->free], 3 token chunks of 428.

v2: interleaved schedule. Per-chunk DDI scans overlap the next chunk's MDM
and the expert matmuls of earlier chunks; gating is batched across all 12
token tiles (2 act-table loads instead of 36); expert path runs in bf16
with all 4 experts' weights resident in SBUF; expert combine uses a
PE row-broadcast of the gates.
"""
import numpy as np
from contextlib import ExitStack

import concourse.bass as bass
import concourse.tile as tile
from concourse import bacc, mybir
from concourse.bass_utils import run_bass_kernel_spmd
from concourse.masks import make_identity

F32 = mybir.dt.float32
F32R = mybir.dt.float32r
BF16 = mybir.dt.bfloat16
AF = mybir.ActivationFunctionType
ALU = mybir.AluOpType
AX = mybir.AxisListType

B, L, C, P_OUT = 32, 336, 321, 96
E, H = 4, 2048
NCORE = 8
BPC = B // NCORE            # 4 batches per core
T = C * BPC                 # 1284 tokens per core
NCH = 3
TCH = T // NCH              # 428 tokens per chunk
CCH = C // NCH              # 107 c's per chunk
PATCH, NPAT = 12, 28
FOLD, FW = 10, 44           # fold groups x width per chunk
PF = FOLD * PATCH           # 120
INV = float(1.0 / np.sqrt(1.0 + 1e-5))
ALPHA = 10.0
D0, D1, D2 = 42, 84, 168    # MDM scale dims (L//8, L//4, L//2)
NTT = NCH * 4               # 12 token tiles of <=128 for gating

_CACHE = {}


def _pad_k(w):
    """[k, m] -> [ceil(k/128), 128, m] zero-padded along k."""
    k, m = w.shape
    kt = -(-k // 128)
    out = np.zeros((kt, 128, m), np.float32)
    out.reshape(kt * 128, m)[:k] = w
    return out


def _bias_cols(b):
    """[m] -> [128, ceil(m/128)] (column mt = partitions of m-tile mt)."""
    m = b.shape[0]
    mt = -(-m // 128)
    out = np.zeros((mt * 128,), np.float32)
    out[:m] = b
    return np.ascontiguousarray(out.reshape(mt, 128).T)


def build_host_weights(inp):
    """Preprocess weights into device layouts (shared across cores)."""
    import ml_dtypes
    w = {}
    for name, src_w, src_b in (
        ("mdm", inp["mdm_bn_w"], inp["mdm_bn_b"]),
        ("ddi", inp["ddi_bn_w"], inp["ddi_bn_b"]),
    ):
        s = (np.asarray(src_w, np.float64) * INV).astype(np.float32).reshape(C, L).T
        t = np.asarray(src_b, np.float32).reshape(C, L).T
        w[f"{name}_s"] = _pad_k(s)
        w[f"{name}_t"] = _pad_k(t)
    pm = np.zeros((L, D0 + D1 + D2), np.float32)
    for i in range(D0):
        pm[8 * i : 8 * i + 8, i] = 0.125
    for i in range(D1):
        pm[4 * i : 4 * i + 4, D0 + i] = 0.25
    for i in range(D2):
        pm[2 * i : 2 * i + 2, D0 + D1 + i] = 0.5
    w["poolmat"] = _pad_k(pm)
    for i in range(3):
        w[f"mw1_{i}"] = _pad_k(np.asarray(inp[f"m{i}w1"], np.float32).T)
        w[f"mw2_{i}"] = _pad_k(np.asarray(inp[f"m{i}w2"], np.float32).T)
        w[f"mb1_{i}"] = _bias_cols(np.asarray(inp[f"m{i}b1"], np.float32))
        w[f"mb2_{i}"] = _bias_cols(np.asarray(inp[f"m{i}b2"], np.float32))
    w["gatew"] = _pad_k(np.asarray(inp["gate_w"], np.float32).T)     # [3,128,4]
    w["gateb"] = np.asarray(inp["gate_b"], np.float32).reshape(1, 4)
    bd = np.kron(np.eye(FOLD, dtype=np.float32),
                 np.asarray(inp["agg_w"], np.float32).T)
    w["bd"] = _pad_k(bd)                                             # [1,128,120]
    w["aggb"] = _bias_cols(np.tile(np.asarray(inp["agg_b"], np.float32), FOLD))
    # n1 scale/shift in folded layout: [128, NCH*FW], row g*12+j, col ch*FW+c
    n1s = (np.asarray(inp["ddi_n1_w"], np.float64) * INV).astype(np.float32).reshape(C, PATCH)
    n1t = np.asarray(inp["ddi_n1_b"], np.float32).reshape(C, PATCH)
    fs = np.zeros((128, NCH * FW), np.float32)
    ft = np.zeros((128, NCH * FW), np.float32)
    for ch in range(NCH):
        for g in range(FOLD):
            for c in range(FW):
                tl = g * FW + c
                if tl >= TCH:
                    continue
                cg = (ch * TCH + tl) // BPC
                fs[g * PATCH : (g + 1) * PATCH, ch * FW + c] = n1s[cg]
                ft[g * PATCH : (g + 1) * PATCH, ch * FW + c] = n1t[cg]
    w["n1sf"], w["n1tf"] = fs, ft
    ew1 = np.asarray(inp["ew1"], np.float32)                         # [E,H,L]
    w["ew1"] = np.stack([_pad_k(ew1[e].T) for e in range(E)]).astype(
        ml_dtypes.bfloat16)                                          # [E,3,128,H]
    w["eb1"] = np.stack([_bias_cols(np.asarray(inp["eb1"], np.float32)[e])
                         for e in range(E)])                         # [E,128,16]
    ew2 = np.asarray(inp["ew2"], np.float32)                         # [E,P,H]
    w["ew2"] = np.stack([np.ascontiguousarray(ew2[e].T.reshape(16, 128, P_OUT))
                         for e in range(E)]).astype(ml_dtypes.bfloat16)
    w["eb2"] = np.stack([_bias_cols(np.asarray(inp["eb2"], np.float32)[e])
                         for e in range(E)])                         # [E,128,1]
    return w


def build_nc(debug=False):
    nc = bacc.Bacc("TRN2", target_bir_lowering=False, debug=False)
    d = {}

    def din(name, shape, dt=F32):
        d[name] = nc.dram_tensor(name, list(shape), dt, kind="ExternalInput")

    din("xT", (3, 128, T))
    for n in ("mdm_s", "mdm_t", "ddi_s", "ddi_t"):
        din(n, (3, 128, C))
    din("poolmat", (3, 128, D0 + D1 + D2))
    din("mw1_0", (1, 128, D0)); din("mw2_0", (1, 128, D1))
    din("mw1_1", (1, 128, D1)); din("mw2_1", (1, 128, D2))
    din("mw1_2", (2, 128, D2)); din("mw2_2", (2, 128, L))
    din("mb1_0", (128, 1)); din("mb2_0", (128, 1))
    din("mb1_1", (128, 1)); din("mb2_1", (128, 2))
    din("mb1_2", (128, 2)); din("mb2_2", (128, 3))
    din("gatew", (3, 128, 4))
    din("gateb", (1, 4))
    din("bd", (1, 128, PF), F32R)
    din("aggb", (128, 1))
    din("n1sf", (128, NCH * FW)); din("n1tf", (128, NCH * FW))
    din("ew1", (E, 3, 128, H), BF16)
    din("eb1", (E, 128, 16))
    din("ew2", (E, 16, 128, P_OUT), BF16)
    din("eb2", (E, 128, 1))
    out_d = nc.dram_tensor("outT", [P_OUT, T], F32, kind="ExternalOutput")
    dbg = {}
    if debug:
        for n, shape, dt_ in (("dbg_te", (3, 128, T), F32),
                              ("dbg_ddi", (3, 128, T), BF16),
                              ("dbg_gates", (4, T), F32)):
            dbg[n] = nc.dram_tensor(n, list(shape), dt_, kind="ExternalOutput")

    with tile.TileContext(nc) as tc, ExitStack() as ctx:
        const = ctx.enter_context(tc.tile_pool(name="const", bufs=1))
        io_p = ctx.enter_context(tc.tile_pool(name="io", bufs=1))
        fold_p = ctx.enter_context(tc.tile_pool(name="fold", bufs=2))
        wk = ctx.enter_context(tc.tile_pool(name="wk", bufs=1))
        sc = ctx.enter_context(tc.tile_pool(name="sc", bufs=2))
        hid_p = ctx.enter_context(tc.tile_pool(name="hid", bufs=1))
        ps_mdm = ctx.enter_context(tc.tile_pool(name="psmdm", bufs=2, space="PSUM"))
        ps_sm = ctx.enter_context(tc.tile_pool(name="pssm", bufs=2, space="PSUM"))
        ps_mm1 = ctx.enter_context(tc.tile_pool(name="psmm1", bufs=2, space="PSUM"))
        ps_mm2 = ctx.enter_context(tc.tile_pool(name="psmm2", bufs=2, space="PSUM"))
        dram = ctx.enter_context(tc.tile_pool(name="dram", bufs=1, space="DRAM"))

        # ---- input on the scalar queue (idle until first gelu), consts on
        # sync, ddi-side consts on gpsimd ----
        xfull = io_p.tile([128, 3, T], F32, name="xfull")
        nc.scalar.dma_start(xfull[:], d["xT"].ap()[:].rearrange("k p t -> p k t"))
        cw = {}
        for n in ("mdm_s", "mdm_t"):
            cw[n] = const.tile([128, 3, C], F32, name=n)
            nc.sync.dma_start(cw[n][:], d[n].ap()[:].rearrange("k p c -> p k c"))
        pool_w = const.tile([128, 3, D0 + D1 + D2], F32, name="poolmat")
        nc.sync.dma_start(pool_w[:], d["poolmat"].ap()[:].rearrange("k p m -> p k m"))
        for n in ("mw1_0", "mw2_0", "mw1_1", "mw2_1", "mw1_2", "mw2_2"):
            kt = d[n].shape[0]
            cw[n] = const.tile([128, kt, d[n].shape[2]], F32, name=n)
            nc.sync.dma_start(cw[n][:], d[n].ap()[:].rearrange("k p m -> p k m"))
        for n in ("mb1_0", "mb2_0", "mb1_1", "mb2_1", "mb1_2", "mb2_2"):
            cw[n] = const.tile(list(d[n].shape), F32, name=n)
            nc.sync.dma_start(cw[n][:], d[n].ap()[:])
        gate_w = const.tile([128, 3, 4], F32, name="gatew")
        nc.sync.dma_start(gate_w[:], d["gatew"].ap()[:].rearrange("k p m -> p k m"))
        gateb_row = const.tile([1, 4], F32, name="gateb_row")
        nc.sync.dma_start(gateb_row[:], d["gateb"].ap()[:])
        for n in ("ddi_s", "ddi_t"):
            cw[n] = const.tile([128, 3, C], F32, name=n)
            nc.gpsimd.dma_start(cw[n][:], d[n].ap()[:].rearrange("k p c -> p k c"))
        cw["bd"] = const.tile([128, 1, PF], F32R, name="bd")
        nc.gpsimd.dma_start(cw["bd"][:], d["bd"].ap()[:].rearrange("k p m -> p k m"))
        for n in ("aggb", "n1sf", "n1tf"):
            cw[n] = const.tile(list(d[n].shape), F32, name=n)
            nc.gpsimd.dma_start(cw[n][:], d[n].ap()[:])
        gateb_bc = const.tile([128, 4], F32, name="gateb_bc")
        nc.gpsimd.partition_broadcast(gateb_bc[:], gateb_row[:])
        negalpha = const.tile([128, 1], F32, name="negalpha")
        nc.vector.memset(negalpha[:], -ALPHA)
        ident = const.tile([128, 128], F32, name="ident")
        make_identity(nc, ident[:])
        ones96 = const.tile([128, P_OUT], F32, name="ones96")
        nc.vector.memset(ones96[:], 1.0)
        # expert weights: all resident; DMAs are emitted after MDM(0) so the
        # chunk-0 dscr/gather DMAs aren't queued behind them on sync
        ew1_t, eb1_t, ew2_t, eb2_t = [], [], [], []
        for e in range(E):
            ew1_t.append(const.tile([128, 3, H], BF16, name=f"ew1_{e}"))
            eb1_t.append(const.tile([128, 16], F32, name=f"eb1_{e}"))
            ew2_t.append(const.tile([128, 16, P_OUT], BF16, name=f"ew2_{e}"))
            eb2_t.append(const.tile([128, 1], F32, name=f"eb2_{e}"))

        def load_expert_weights():
            for e in range(E):
                nc.sync.dma_start(ew1_t[e][:], d["ew1"].ap()[e].rearrange("k p m -> p k m"))
                nc.sync.dma_start(eb1_t[e][:], d["eb1"].ap()[e])
                nc.sync.dma_start(ew2_t[e][:], d["ew2"].ap()[e].rearrange("k p m -> p k m"))
                nc.sync.dma_start(eb2_t[e][:], d["eb2"].ap()[e])

        # ---- persistent tiles ----
        ddi_out = io_p.tile([128, 3, T], BF16, name="ddi_out")
        nc.vector.memset(ddi_out[:, 2, :].bitcast(F32), 0.0)
        gates_fm4 = io_p.tile([4, T], F32, name="gates_fm4")
        gates_fm = io_p.tile([128, T], F32, name="gates_fm")
        z_all = io_p.tile([128, NTT, 4], F32, name="z_all")
        nc.vector.memset(z_all[:], 0.0)
        acc = [io_p.tile([P_OUT, TCH], F32, name=f"acc{ch}") for ch in range(NCH)]
        dscr = dram.tile([L, T], F32R, name="dscr")
        oscr = [dram.tile([L, TCH], BF16, name=f"oscr{ch}") for ch in range(NCH)]

        # MDM persistent-by-name work tiles (memset tails once; reused per chunk)
        s0 = wk.tile([128, TCH], F32, name="s0")
        s1 = wk.tile([128, TCH], F32, name="s1")
        s2 = wk.tile([128, 2, TCH], F32, name="s2")
        a0 = wk.tile([128, TCH], F32, name="a0")
        cur1 = wk.tile([128, TCH], F32, name="cur1")
        cur2 = wk.tile([128, 2, TCH], F32, name="cur2")
        a2 = wk.tile([128, 2, TCH], F32, name="a2")
        te = wk.tile([128, 3, TCH], F32, name="te")
        nc.vector.memset(s0[:], 0.0)
        nc.vector.memset(s1[:], 0.0)
        nc.vector.memset(s2[:, 1, :], 0.0)
        nc.vector.memset(a0[:], 0.0)
        nc.vector.memset(cur1[:], 0.0)
        nc.vector.memset(cur2[:, 1, :], 0.0)
        nc.vector.memset(a2[:, 1, :], 0.0)
        nc.vector.memset(te[:, 2, :], 0.0)

        # =========== stage builders ===========
        def mdm_stages(ch):
            tsl = slice(ch * TCH, (ch + 1) * TCH)
            csl = slice(ch * CCH, (ch + 1) * CCH)
            st = []

            def bn_into(dst, src, sname, tname):
                # Pool engine: keeps the DVE free for the scan chains
                for k in range(3):
                    sB = cw[sname][:, k, csl].unsqueeze(2).broadcast_to([128, CCH, BPC])
                    tB = cw[tname][:, k, csl].unsqueeze(2).broadcast_to([128, CCH, BPC])
                    xv = src[:, k, :].rearrange("p (c b) -> p c b", b=BPC)
                    hv = dst[:, k, :].rearrange("p (c b) -> p c b", b=BPC)
                    nc.vector.tensor_tensor(hv, xv, sB, op=ALU.mult)
                    nc.vector.tensor_tensor(hv, hv, tB, op=ALU.add)

            h = wk.tile([128, 3, TCH], F32, name="h")

            def st_bn():
                bn_into(h, xfull[:, :, tsl], "mdm_s", "mdm_t")
            st.append(st_bn)

            for m0, m1 in ((0, D0), (D0, D0 + D1), (D0 + D1, D0 + D1 + 128),
                           (D0 + D1 + 128, D0 + D1 + D2)):
                def st_pool(m0=m0, m1=m1):
                    if m0 == 0:
                        dst = s0[:D0, :]
                    elif m0 == D0:
                        dst = s1[:D1, :]
                    elif m0 == D0 + D1:
                        dst = s2[:128, 0, :]
                    else:
                        dst = s2[: D2 - 128, 1, :]
                    pp = ps_mdm.tile([128, TCH], F32, name="pmdm")
                    for k in range(3):
                        nc.tensor.matmul(pp[: m1 - m0, :], pool_w[:, k, m0:m1],
                                         h[:, k, :], start=(k == 0), stop=(k == 2))
                    nc.vector.tensor_copy(dst, pp[: m1 - m0, :])
                st.append(st_pool)

            def st_mlp0():
                pa = ps_mdm.tile([128, TCH], F32, name="pmdm")
                nc.tensor.matmul(pa[:D0, :], cw["mw1_0"][:, 0, :], s0[:], start=True, stop=True)
                nc.scalar.activation(a0[:D0, :], pa[:D0, :], AF.Gelu, bias=cw["mb1_0"][:D0, 0:1])
                pt = ps_mdm.tile([128, TCH], F32, name="pmdm")
                nc.tensor.matmul(pt[:D1, :], cw["mw2_0"][:, 0, :], a0[:], start=True, stop=True)
                nc.vector.scalar_tensor_tensor(cur1[:D1, :], pt[:D1, :], cw["mb2_0"][:D1, 0:1],
                                               s1[:D1, :], op0=ALU.add, op1=ALU.add)
            st.append(st_mlp0)

            def st_mlp1():
                pa = ps_mdm.tile([128, TCH], F32, name="pmdm")
                nc.tensor.matmul(pa[:D1, :], cw["mw1_1"][:, 0, :], cur1[:], start=True, stop=True)
                a1 = a0
                nc.scalar.activation(a1[:D1, :], pa[:D1, :], AF.Gelu, bias=cw["mb1_1"][:D1, 0:1])
                for mt, (p0, pk) in enumerate(((0, 128), (128, D2 - 128))):
                    pt = ps_mdm.tile([128, TCH], F32, name="pmdm")
                    nc.tensor.matmul(pt[:pk, :], cw["mw2_1"][:, 0, p0 : p0 + pk], a1[:],
                                     start=True, stop=True)
                    nc.vector.scalar_tensor_tensor(cur2[:pk, mt, :], pt[:pk, :],
                                                   cw["mb2_1"][:pk, mt : mt + 1],
                                                   s2[:pk, mt, :], op0=ALU.add, op1=ALU.add)
            st.append(st_mlp1)

            def st_mlp2a():
                for mt, (p0, pk) in enumerate(((0, 128), (128, D2 - 128))):
                    pa = ps_mdm.tile([128, TCH], F32, name="pmdm")
                    for k in range(2):
                        nc.tensor.matmul(pa[:pk, :], cw["mw1_2"][:, k, p0 : p0 + pk],
                                         cur2[:, k, :], start=(k == 0), stop=(k == 1))
                    nc.scalar.activation(a2[:pk, mt, :], pa[:pk, :], AF.Gelu,
                                         bias=cw["mb1_2"][:pk, mt : mt + 1])
            st.append(st_mlp2a)

            def st_mlp2b():
                for mt in range(3):
                    pk = 128 if mt < 2 else L - 256
                    pt = ps_mdm.tile([128, TCH], F32, name="pmdm")
                    for k in range(2):
                        nc.tensor.matmul(pt[:pk, :], cw["mw2_2"][:, k, 128 * mt : 128 * mt + pk],
                                         a2[:, k, :], start=(k == 0), stop=(k == 1))
                    nc.vector.scalar_tensor_tensor(te[:pk, mt, :], pt[:pk, :],
                                                   cw["mb2_2"][:pk, mt : mt + 1],
                                                   h[:pk, mt, :], op0=ALU.add, op1=ALU.add)
                if debug:
                    for k in range(3):
                        nc.sync.dma_start(dbg["dbg_te"].ap()[k, :, tsl], te[:, k, :])
            st.append(st_mlp2b)

            def st_logits():
                for mt in range(4):
                    t0_ = mt * 128
                    tw = min(128, TCH - t0_)
                    pz = ps_sm.tile([128, 128], F32, name="pssm")
                    for k in range(3):
                        nc.tensor.matmul(pz[:tw, :4], te[:, k, t0_ : t0_ + tw],
                                         gate_w[:, k, :], start=(k == 0), stop=(k == 2))
                    nc.vector.tensor_tensor(z_all[:tw, ch * 4 + mt, :], pz[:tw, :4],
                                            gateb_bc[:tw, :], op=ALU.add)
            st.append(st_logits)

            def st_ddibn():
                dt_ = h  # reuse h buffer: h is dead after st_mlp2b consumed it
                bn_into(dt_, te, "ddi_s", "ddi_t")
                for k, pk in ((0, 128), (1, 128), (2, L - 256)):
                    nc.sync.dma_start(dscr[128 * k : 128 * k + pk, tsl],
                                      dt_[:pk, k, :].bitcast(F32R))
            st.append(st_ddibn)
            return st

        def scan_stages(ch):
            """DDI fold + sequential scan + unfold for one chunk."""
            tsl = slice(ch * TCH, (ch + 1) * TCH)
            fsl = slice(ch * FW, (ch + 1) * FW)
            dF = fold_p.tile([128, NPAT, FW], F32R, name="dF")
            xpn = fold_p.tile([128, NPAT, FW], F32R, name="xpn")
            outB = fold_p.tile([128, NPAT, FW], BF16, name="outB")
            # per-chunk scratch: chunks' scans run concurrently
            t1 = wk.tile([128, FW], F32R, name=f"t1_{ch}")
            t2g = wk.tile([128, FW], F32R, name=f"t2g_{ch}")
            st = []

            def st_fold():
                # zero tails that matmuls/vector ops read but gathers don't
                # write (partition bases must be 32-aligned -> full memsets)
                nc.vector.memset(dF[:].bitcast(F32), 0.0)
                nc.vector.memset(t1[:].bitcast(F32), 0.0)
                nc.vector.memset(xpn[:].bitcast(F32), 0.0)
                for g in range(FOLD):
                    cw_ = min(FW, TCH - g * FW)
                    src = dscr[:, ch * TCH + g * FW : ch * TCH + g * FW + cw_].rearrange(
                        "(s j) c -> j s c", j=PATCH)
                    nc.sync.dma_start(dF[g * PATCH : (g + 1) * PATCH, :, :cw_], src)
            st.append(st_fold)

            def st_xpn():
                n1sB = cw["n1sf"][:PF, fsl].unsqueeze(1).broadcast_to([PF, NPAT, FW])
                n1tB = cw["n1tf"][:PF, fsl].unsqueeze(1).broadcast_to([PF, NPAT, FW])
                nc.vector.tensor_tensor(xpn[:PF, :, :], dF[:PF, :, :], n1sB, op=ALU.mult)
                nc.vector.tensor_tensor(xpn[:PF, :, :], xpn[:PF, :, :], n1tB, op=ALU.add)
                nc.vector.tensor_copy(outB[:PF, 0, :], dF[:PF, 0, :])
            st.append(st_xpn)

            for s in range(1, NPAT):
                def st_step(s=s):
                    if s == 1:
                        rhs = xpn[:, 0, :]
                    else:
                        nc.vector.tensor_tensor(t1[:PF, :], t2g[:PF, :],
                                                cw["n1sf"][:PF, fsl], op=ALU.mult)
                        nc.vector.tensor_tensor(t1[:PF, :], t1[:PF, :],
                                                xpn[:PF, s - 1, :], op=ALU.add)
                        rhs = t1[:, :]
                    pd = ps_sm.tile([128, 128], F32, name="pssm")
                    nc.tensor.matmul(pd[:PF, :FW], cw["bd"][:, 0, :], rhs,
                                     start=True, stop=True)
                    nc.scalar.activation(t2g[:PF, :], pd[:PF, :FW], AF.Gelu,
                                         bias=cw["aggb"][:PF, 0:1])
                    nc.vector.tensor_tensor(outB[:PF, s, :], t2g[:PF, :],
                                            dF[:PF, s, :], op=ALU.add)
                st.append(st_step)

            def st_unfold():
                for g in range(FOLD):
                    cw_ = min(FW, TCH - g * FW)
                    dst = oscr[ch][:, g * FW : g * FW + cw_].rearrange(
                        "(s j) c -> j s c", j=PATCH)
                    nc.gpsimd.dma_start(dst, outB[g * PATCH : (g + 1) * PATCH, :, :cw_])
                for k, pk in ((0, 128), (1, 128), (2, L - 256)):
                    nc.gpsimd.dma_start(ddi_out[:pk, k, tsl],
                                        oscr[ch][128 * k : 128 * k + pk, :])
                if debug:
                    for k in range(3):
                        nc.sync.dma_start(dbg["dbg_ddi"].ap()[k, :, tsl],
                                          ddi_out[:, k, tsl])
            st.append(st_unfold)
            return st

        def gating_stages():
            """Batched noisy-top-k gating over all 12 token tiles."""
            st = []

            def sc3(name):
                return sc.tile([128, NTT, 4], F32, name=name)

            def sc1(name):
                return sc.tile([128, NTT], F32, name=name)

            def bcast(t):
                return t[:].unsqueeze(2).broadcast_to([128, NTT, 4])

            gtm = io_p.tile([128, NTT, 4], F32, name="gtm")

            def st_vec1():
                z3 = z_all[:, :, :]
                m1 = sc1("m1")
                nc.vector.reduce_max(m1[:], z3, axis=AX.X)
                ismax = sc3("ismax")
                nc.vector.tensor_tensor(ismax[:], z3, bcast(m1), op=ALU.is_ge)
                cnt = sc1("cnt")
                nc.vector.reduce_sum(cnt[:], ismax[:], axis=AX.X)
                masked = sc3("masked")
                nc.vector.scalar_tensor_tensor(masked[:], ismax[:], -1e30, z3,
                                               op0=ALU.mult, op1=ALU.add)
                m2 = sc1("m2")
                nc.vector.reduce_max(m2[:], masked[:], axis=AX.X)
                ge2 = sc1("ge2")
                nc.vector.tensor_scalar(ge2[:], cnt[:], 2.0, None, op0=ALU.is_ge)
                d12 = sc1("d12")
                nc.vector.tensor_tensor(d12[:], m1[:], m2[:], op=ALU.subtract)
                kth = sc1("kth")
                nc.vector.tensor_tensor(kth[:], d12[:], ge2[:], op=ALU.mult)
                nc.vector.tensor_tensor(kth[:], kth[:], m2[:], op=ALU.add)
                zs = sc3("zs")
                nc.vector.tensor_tensor(zs[:], z3, bcast(m1), op=ALU.subtract)
                ez = sc3("ez")
                nc.scalar.activation(ez[:], zs[:], AF.Exp)
                se = sc1("se")
                nc.vector.reduce_sum(se[:], ez[:], axis=AX.X)
                rse = sc1("rse")
                nc.vector.reciprocal(rse[:], se[:])
                s_ = sc3("s_")
                nc.vector.tensor_tensor(s_[:], ez[:], bcast(rse), op=ALU.mult)
                # softmax(dec) computed without ln (exp-only, one act table):
                # masked experts: exp(A*ln(1+s)) = (1+s)^10 as a polynomial;
                # top-2 experts:  exp(A*(e^s - 1)) = exp(10*e^s - 10).
                # Values stay in [1, e^17.2] -> no max-subtraction needed.
                p1t = sc3("p1t")
                nc.vector.tensor_scalar(p1t[:], s_[:], 1.0, None, op0=ALU.add)
                p2t = sc3("p2t")
                nc.vector.tensor_tensor(p2t[:], p1t[:], p1t[:], op=ALU.mult)
                p4t = sc3("p4t")
                nc.vector.tensor_tensor(p4t[:], p2t[:], p2t[:], op=ALU.mult)
                p8t = sc3("p8t")
                nc.vector.tensor_tensor(p8t[:], p4t[:], p4t[:], op=ALU.mult)
                p10 = sc3("p10")
                nc.vector.tensor_tensor(p10[:], p8t[:], p2t[:], op=ALU.mult)
                u_ = sc3("u_")
                nc.scalar.activation(u_[:], s_[:], AF.Exp)
                w_ = sc3("w_")
                nc.scalar.activation(w_[:], u_[:], AF.Exp, bias=negalpha[:, 0:1],
                                     scale=ALPHA)
                mask = sc3("mask")
                nc.vector.tensor_tensor(mask[:], z3, bcast(kth), op=ALU.is_lt)
                fd = sc3("fd")
                nc.vector.tensor_tensor(fd[:], p10[:], w_[:], op=ALU.subtract)
                f_ = sc3("f_")
                nc.vector.tensor_tensor(f_[:], mask[:], fd[:], op=ALU.mult)
                nc.vector.tensor_tensor(f_[:], f_[:], w_[:], op=ALU.add)
                sx = sc1("sx")
                nc.vector.reduce_sum(sx[:], f_[:], axis=AX.X)
                rsx = sc1("rsx")
                nc.vector.reciprocal(rsx[:], sx[:])
                nc.vector.tensor_tensor(gtm[:], f_[:], bcast(rsx), op=ALU.mult)
            st.append(st_vec1)

            def st_tr():
                for ti in range(NTT):
                    ch, mt = divmod(ti, 4)
                    t0_ = ch * TCH + mt * 128
                    tw = min(128, TCH - mt * 128)
                    ptr = ps_sm.tile([128, 128], F32, name="pssm")
                    nc.tensor.transpose(ptr[:4, :tw], gtm[:tw, ti, :], ident[:tw, :tw])
                    nc.vector.tensor_copy(gates_fm4[0:4, t0_ : t0_ + tw], ptr[:4, :tw])
                nc.sync.dma_start(gates_fm[0:128:32, :], gates_fm4[0:4, :])
                if debug:
                    nc.sync.dma_start(dbg["dbg_gates"].ap()[:], gates_fm[0:128:32, :])
            st.append(st_tr)
            return st

        def expert_stages(ch):
            tsl = slice(ch * TCH, (ch + 1) * TCH)
            st = []
            for e in range(E):
                hid = hid_p.tile([128, 16, TCH], BF16, name="hid")
                for mtg in range(8):
                    def st_mm1(e=e, hid=hid, mtg=mtg):
                        for mt in range(2 * mtg, 2 * mtg + 2):
                            p1 = ps_mm1.tile([128, TCH], F32, name="pmm1")
                            for k in range(3):
                                nc.tensor.matmul(p1[:], ew1_t[e][:, k, 128 * mt : 128 * (mt + 1)],
                                                 ddi_out[:, k, tsl], start=(k == 0), stop=(k == 2))
                            nc.scalar.activation(hid[:, mt, :], p1[:], AF.Gelu,
                                                 bias=eb1_t[e][:, mt : mt + 1])
                    st.append(st_mm1)

                def st_mm2(e=e, hid=hid):
                    p2 = ps_mm2.tile([P_OUT, TCH], F32, name="pmm2")
                    for kt in range(16):
                        nc.tensor.matmul(p2[:], ew2_t[e][:, kt, :], hid[:, kt, :],
                                         start=(kt == 0), stop=(kt == 15))
                    gps = ps_mdm.tile([128, TCH], F32, name="pmdm")
                    nc.tensor.matmul(gps[:P_OUT, :], ones96[32 * e : 32 * e + 1, :],
                                     gates_fm[32 * e : 32 * e + 1, tsl],
                                     start=True, stop=True, tile_position=(32 * e, 0))
                    gbc = sc.tile([P_OUT, TCH], F32, name="gbc")
                    nc.vector.tensor_copy(gbc[:], gps[:P_OUT, :])
                    if e == 0:
                        nc.vector.scalar_tensor_tensor(acc[ch][:], p2[:], eb2_t[e][:P_OUT, 0:1],
                                                       gbc[:], op0=ALU.add, op1=ALU.mult)
                    else:
                        tmp = sc.tile([P_OUT, TCH], F32, name="etmp")
                        nc.vector.scalar_tensor_tensor(tmp[:], p2[:], eb2_t[e][:P_OUT, 0:1],
                                                       gbc[:], op0=ALU.add, op1=ALU.mult)
                        nc.vector.tensor_tensor(acc[ch][:], acc[ch][:], tmp[:], op=ALU.add)
                    if e == E - 1:
                        nc.sync.dma_start(out_d.ap()[:, tsl], acc[ch][:])
                st.append(st_mm2)
            return st

        def weave(a, b):
            """Interleave two stage lists proportionally (a paces, b spreads)."""
            out, i, j = [], 0, 0
            while i < len(a) or j < len(b):
                fa = i / len(a) if a else 1.0
                fb = j / len(b) if b else 1.0
                if i < len(a) and (j >= len(b) or fa <= fb):
                    out.append(a[i]); i += 1
                else:
                    out.append(b[j]); j += 1
            return out

        def run_st(stages):
            for s in stages:
                s()

        def interleave2(a, b):
            """Alternate elements of two lists (chains progress concurrently)."""
            out = []
            for i in range(max(len(a), len(b))):
                if i < len(a):
                    out.append(a[i])
                if i < len(b):
                    out.append(b[i])
            return out

        # =========== schedule ===========
        # Scans are staggered two-deep: scan(ch) overlaps MDM(ch+1) and the
        # tail of scan(ch-1); scan(2) overlaps the chunk-0 expert matmuls.
        s0_ = scan_stages(0)
        s1_ = scan_stages(1)
        s2_ = scan_stages(2)
        run_st(mdm_stages(0))
        load_expert_weights()
        run_st(weave(mdm_stages(1), s0_[:16]))
        # keep most of scan(1)'s chain out of P3: its matmuls trickle at the
        # head of the in-order PE queue and would stall the experts behind
        # them; ride the chains inside the expert phases instead.
        run_st(weave(mdm_stages(2) + gating_stages(),
                     interleave2(s0_[16:], s1_[:4])))
        # scan(ch) must be fully emitted before expert chunk ch reads
        # ddi_out (tile deps are program-order based).
        run_st(weave(expert_stages(0), weave(s1_[4:], s2_[:20])))
        run_st(weave(expert_stages(1), s2_[20:]))
        run_st(expert_stages(2))

    nc.compile()
    return nc


def _get_nc(debug=False):
    key = "dbg" if debug else "std"
    if key not in _CACHE:
        _CACHE[key] = build_nc(debug)
    return _CACHE[key]


def make_in_maps(inputs):
    w = build_host_weights(inputs)
    x = np.asarray(inputs["x"], np.float32)
    in_maps = []
    for m in range(NCORE):
        xm = x[m * BPC : (m + 1) * BPC]                   # [4, L, C]
        xT = np.zeros((3, 128, T), np.float32)
        xT.reshape(384, T)[:L] = xm.transpose(1, 2, 0).reshape(L, T)
        im = {"xT": xT}
        im.update(w)
        in_maps.append(im)
    return in_maps


def _gather_out(results):
    out = np.empty((B, P_OUT, C), np.float32)
    for m in range(NCORE):
        o = results[m]["outT"]                            # [96, 1284]
        out[m * BPC : (m + 1) * BPC] = o.reshape(P_OUT, C, BPC).transpose(2, 0, 1)
    return out


def run(inputs, debug=False, **kw):
    nc = _get_nc(debug=debug)
    in_maps = make_in_maps(inputs)
    res = run_bass_kernel_spmd(nc, in_maps, core_ids=list(range(NCORE)), **kw)
    return res


def kernel(**inputs) -> np.ndarray:
    res = run(inputs, debug=False)
    return _gather_out(res.results)
